# revision 48
# baseline (speedup 1.0000x reference)
"""GCN VGAE encoder (3x GCNConv) on 8 Trainium2 NeuronCores — v2.

Strategy: shard nodes across 8 cores, partition edges by destination
tile, replicate weights, AllGather the projected node-feature table
between layers, gather rows per edge with dma_gather (4 SWDGE queues,
one per table quarter).

v2 math: all of A_hat's normalization is folded into per-edge weights
w_e = dinv[src]*dinv[dst] (self-loops are explicit edges with w =
dinv^2), so
    gcn1: h   = relu(sum_e w_e * (x W1)[src] + b1)        per dst
    gcn2: out = (sum_e w_e * h[src]) @ [W_mu|W_log] + b   per dst
Aggregation runs TRANSPOSED on PE: psaT[feat, dst] += matmul(
lhsT=gathered[edge, feat], rhs=onehot[edge, dst]) where onehot =
(iota == dstloc) * w is a single fused DVE op. PSUM drains go through
the (otherwise idle) Activation engine, so the DVE one-hot stream and
the PE matmul stream never block on per-tile epilogues.
"""

import os

import numpy as np

P = 128


def _ceil_div(a, b):
    return -(-a // b)


class _Plan:
    """Host-side edge partitioning shared by all cores (SPMD => one
    common chunk structure = max over cores, padded)."""

    def __init__(self, n, n_cores, cpc, src, dst, max_slice_rows=32000):
        assert n % n_cores == 0
        self.n = n
        self.n_cores = n_cores
        self.cpc = cpc                     # chunks per dma_gather call
        self.S = n // n_cores              # nodes per core
        self.T = _ceil_div(self.S, P)      # dst tiles per core
        self.SPAD = self.T * P
        # tables are packed two nodes per 256B row; edges stream in 4
        # groups (node half x parity) so each chunk reads one uniform
        # 64-col slice of its gathered pair-rows. int16 gather idx =>
        # pair rows per table slice <= 32767.
        assert n % 4 == 0 and n // 4 <= max_slice_rows
        self.NQ = 4
        self.QRP = n // 4                  # pair-rows per table slice

        # normalization weights; self-loops appended as explicit edges
        deg = np.bincount(dst, minlength=n)
        dinv = 1.0 / np.sqrt(deg + 1.0)
        loops = np.arange(n, dtype=src.dtype)
        src = np.concatenate([src, loops])
        dst = np.concatenate([dst, loops])
        w = (dinv[src] * dinv[dst]).astype(np.float32)

        core = dst // self.S
        drel = dst - core * self.S
        tt = drel // P
        loc = (drel % P).astype(np.float32)
        half = src // (n // 2)
        q = half * 2 + (src % 2)           # stream = (half, parity)
        qsrc = (src // 2 - half * self.QRP).astype(np.int16)

        T, NQ = self.T, self.NQ
        key = (core * NQ + q) * T + tt
        counts = np.bincount(key, minlength=n_cores * NQ * T).reshape(
            n_cores, NQ, T
        )
        # common run length per (quarter, tile): max over cores, runs are
        # packed back-to-back in the quarter stream (no 128-padding per
        # run; chunks may span adjacent tiles).
        self.rl = counts.max(axis=0)                         # [NQ, T]
        self.run_start = np.zeros((NQ, T), np.int64)
        self.run_start[:, 1:] = np.cumsum(self.rl, axis=1)[:, :-1]
        self.NQE = self.rl.sum(axis=1)                       # edges/quarter
        self.NQC = _ceil_div(self.NQE, P)                    # chunks/quarter
        self.NCH = int(self.NQC.sum())

        # order edges by (core, quarter, tile); rank within group
        sidx = np.lexsort((tt, q, core))
        self.sc = core[sidx]
        self.sq = q[sidx]
        self.st = tt[sidx]
        self.sqsrc = qsrc[sidx]
        self.sloc = loc[sidx]
        self.sw = w[sidx]
        gkey = (self.sc * NQ + self.sq) * T + self.st
        first = np.r_[True, gkey[1:] != gkey[:-1]]
        gstart = np.flatnonzero(first)
        glen = np.diff(np.r_[gstart, len(gkey)])
        self.rank = np.arange(len(gkey)) - np.repeat(gstart, glen)

        # chunk-part (cp) map: device consumes tiles in order; for tile t
        # and quarter q, the run covers chunks j0..j1 of quarter q's
        # stream; each (t, q, j) overlap gets its own dstloc column.
        self.tile_ops = []        # [T] -> list of (q, j, cp_col)
        ncp = 0
        for t in range(T):
            ops = []
            for qq in range(NQ):
                r0 = int(self.run_start[qq, t])
                r1 = r0 + int(self.rl[qq, t])
                if r1 == r0:
                    continue
                for j in range(r0 // P, (r1 - 1) // P + 1):
                    ops.append((qq, j, ncp))
                    ncp += 1
            self.tile_ops.append(ops)
        self.NCP = ncp
        # vectorized cp lookup: cp = cp_base[t] + ops_before[q,t] + (j - j0)
        self.cp_base = np.zeros(T, np.int64)
        run2 = 0
        self.ops_before = np.zeros((NQ, T), np.int64)
        self.j0 = self.run_start // P
        for t in range(T):
            self.cp_base[t] = run2
            acc = 0
            for qq in range(NQ):
                self.ops_before[qq, t] = acc
                if self.rl[qq, t] > 0:
                    r0 = int(self.run_start[qq, t])
                    r1 = r0 + int(self.rl[qq, t])
                    acc += (r1 - 1) // P - r0 // P + 1
            run2 += acc
        assert run2 == ncp

        # gather calls per quarter
        self.ncalls = [_ceil_div(int(c), cpc) for c in self.NQC]
        # idx tensor column offset of each (quarter, call)
        self.call_col0 = {}
        col = 0
        for qq in range(NQ):
            for k in range(self.ncalls[qq]):
                L = min(cpc, int(self.NQC[qq]) - k * cpc)
                self.call_col0[(qq, k)] = (col, L)
                col += L * 8
        self.IDXCOLS = col

    def core_arrays(self, c):
        """Per-core upload tensors: gather idx [128, IDXCOLS] i16,
        dstloc [128, NCP] f32, edge weight [128, NCP] f32."""
        NQ, cpc = self.NQ, self.cpc
        m_core = self.sc == c
        idx_out = np.zeros((P, self.IDXCOLS), np.int16)
        dl = np.full((self.NCP, P), 255.0, np.float32)
        vl = np.zeros((self.NCP, P), np.float32)
        mloc = self.sloc[m_core]
        mq = self.sq[m_core]
        mt = self.st[m_core]
        mrank = self.rank[m_core]
        msrc = self.sqsrc[m_core]
        mw = self.sw[m_core]
        # stream position of each edge within its quarter
        pos = self.run_start[mq, mt] + mrank
        cpcol = (self.cp_base[mt] + self.ops_before[mq, mt]
                 + pos // P - self.j0[mq, mt])
        dl[cpcol, pos % P] = mloc
        vl[cpcol, pos % P] = mw
        for qq in range(NQ):
            mm = mq == qq
            arr = np.zeros(int(self.NQC[qq]) * P, np.int16)
            arr[pos[mm]] = msrc[mm]
            for k in range(self.ncalls[qq]):
                c0, L = self.call_col0[(qq, k)]
                seg = arr[k * cpc * P:(k * cpc + L) * P]
                wrapped = seg.reshape(L * 8, 16).T       # [16, L*8]
                idx_out[:, c0:c0 + L * 8] = np.tile(wrapped, (8, 1))
        return idx_out, dl.T.copy(), vl.T.copy()


def _build(plan, d_in, d_h, d_o):
    """Build the SPMD Bass program (same for every core)."""
    import concourse.mybir as mybir
    import concourse.tile as tile
    from concourse import bacc
    from concourse.masks import make_identity

    F32 = mybir.dt.float32
    BF16 = mybir.dt.bfloat16
    I16 = mybir.dt.int16
    AF = mybir.ActivationFunctionType
    DPAD = 2 * d_h   # two bf16 nodes per 256B table row (gather elem size)
    n, T, NQ, SPAD, S = plan.n, plan.T, plan.NQ, plan.SPAD, plan.S
    QRP = plan.QRP
    cpc = plan.cpc
    n_cores = plan.n_cores

    nqq = int(os.environ.get("GCN_QUEUES", "4"))
    nc = bacc.Bacc("TRN2", target_bir_lowering=False,
                   debug=False, num_devices=n_cores,
                   num_swdge_queues=nqq)

    xT_d = nc.dram_tensor("xT", [d_in, SPAD], BF16, kind="ExternalInput")
    w1_d = nc.dram_tensor("w1", [d_in, d_h], BF16, kind="ExternalInput")
    wc_d = nc.dram_tensor("wcat", [d_h, d_o], F32, kind="ExternalInput")
    b1_d = nc.dram_tensor("b1r", [1, d_h], BF16, kind="ExternalInput")
    bc_d = nc.dram_tensor("bcr", [1, d_o], F32, kind="ExternalInput")
    dl_d = nc.dram_tensor("dstloc", [P, plan.NCP], F32, kind="ExternalInput")
    vl_d = nc.dram_tensor("edgew", [P, plan.NCP], F32, kind="ExternalInput")
    idx_d = nc.dram_tensor("gidx", [P, plan.IDXCOLS], I16, kind="ExternalInput")
    out_d = nc.dram_tensor("out2", [SPAD, d_o], F32, kind="ExternalOutput")

    t1s_own = nc.dram_tensor("t1s_own", [S, d_h], BF16, kind="Internal")
    t1s_full = nc.dram_tensor("t1s_full", [n // 2, DPAD], BF16,
                              kind="Internal", addr_space="Shared")
    t2s_own = nc.dram_tensor("t2s_own", [S, d_h], BF16, kind="Internal")
    t2s_full = nc.dram_tensor("t2s_full", [n // 2, DPAD], BF16,
                              kind="Internal", addr_space="Shared")
    rg = [list(range(n_cores))]
    _nocoll = bool(os.environ.get("GCN_NOCOLL"))

    from contextlib import ExitStack

    with tile.TileContext(nc, num_cores=n_cores) as tc, ExitStack() as st:
        cp = st.enter_context(tc.tile_pool(name="consts", bufs=1))
        xp = st.enter_context(tc.tile_pool(name="x", bufs=3))
        t1p = st.enter_context(tc.tile_pool(name="t1", bufs=3))
        hp = st.enter_context(tc.tile_pool(name="h", bufs=3))
        zp = st.enter_context(tc.tile_pool(name="z", bufs=3))
        zTp = st.enter_context(tc.tile_pool(name="zT", bufs=3))
        op = st.enter_context(tc.tile_pool(name="o", bufs=3))
        ohp = st.enter_context(tc.tile_pool(name="oh", bufs=8))
        gps = [st.enter_context(tc.tile_pool(name=f"g{q}", bufs=4))
               for q in range(NQ)]
        mmp = st.enter_context(tc.tile_pool(name="mm", bufs=3, space="PSUM"))
        aggp = st.enter_context(tc.tile_pool(name="agg", bufs=3,
                                             space="PSUM"))

        # ---- constants ----
        iota_i = cp.tile([P, P], mybir.dt.int32)
        nc.gpsimd.iota(iota_i[:], pattern=[[1, P]], base=0,
                       channel_multiplier=0)
        iota_f = cp.tile([P, P], BF16)
        nc.vector.tensor_copy(iota_f[:], iota_i[:])
        ident = cp.tile([P, P], F32)
        make_identity(nc, ident[:])
        ones_row = cp.tile([1, P], F32)
        nc.gpsimd.memset(ones_row[:], 1.0)
        ones_bf = cp.tile([1, P], BF16)
        nc.gpsimd.memset(ones_bf[:], 1.0)

        w1_sb = cp.tile([d_in, d_h], BF16)
        nc.sync.dma_start(w1_sb[:], w1_d[:, :])
        wc_sb = cp.tile([d_h, d_o], F32)
        nc.sync.dma_start(wc_sb[:], wc_d[:, :])
        b1r = cp.tile([1, d_h], BF16)
        nc.sync.dma_start(b1r[:], b1_d[:, :])
        bcr = cp.tile([1, d_o], F32)
        nc.sync.dma_start(bcr[:], bc_d[:, :])

        dl_sb = cp.tile([P, plan.NCP], F32)
        nc.sync.dma_start(dl_sb[:], dl_d[:, :])
        vl_sb = cp.tile([P, plan.NCP], F32)
        nc.sync.dma_start(vl_sb[:], vl_d[:, :])
        idx_sb = cp.tile([P, plan.IDXCOLS], I16)
        nc.sync.dma_start(idx_sb[:], idx_d[:, :])

        # ---- layer-1 projection: t1s_own = x @ W1 (raw, no scaling) ----
        for t in range(T):
            xt = xp.tile([d_in, P], BF16, tag="x")
            nc.sync.dma_start(xt[:], xT_d[:, t * P:(t + 1) * P])
            psm = mmp.tile([P, P], F32, space="PSUM", tag="mm")
            nc.tensor.matmul(psm[:, :d_h], lhsT=xt[:], rhs=w1_sb[:],
                             start=True, stop=True)
            t1t = t1p.tile([P, d_h], BF16, tag="t1")
            nc.scalar.activation(t1t[:], psm[:, :d_h], AF.Copy)
            r0 = t * P
            r1 = min(S, r0 + P)
            if r1 > r0:
                nc.sync.dma_start(t1s_own[r0:r1, :], t1t[:r1 - r0, :])

        if _nocoll:
            nc.sync.dma_start(t1s_full[0:S // 2, :], t1s_own[:, :])
        else:
            nc.gpsimd.collective_compute(
                "AllGather", mybir.AluOpType.bypass, replica_groups=rg,
                ins=[t1s_own[:, :].opt()], outs=[t1s_full[:, :].opt()])

        def gather128(g_ap, in_ap, idxs_ap, L, queue):
            """dma_gather with 128-byte elements (single bf16 node row out
            of a 256B-stride pair-row table). bass.dma_gather asserts
            elem_size_bytes % 256 == 0 (a transpose-mode restriction), so
            mirror its non-transpose construction directly."""
            gp = nc.gpsimd
            _in = gp.lower_ap_dma(in_ap, for_custom_bir_dma=True)
            return gp.add_instruction(
                mybir.InstDMAGatherAnt(
                    name=nc.get_next_instruction_name(),
                    ins=[*_in, gp.lower_ap(idxs_ap),
                         gp.lower_val_access(gp.to_reg(L * P))],
                    outs=[gp.lower_ap(g_ap)],
                    transpose=False,
                    num_idxs=L * P,
                    elem_size=d_h,
                    stride_bytes_256=1,
                    gen_mode=0,
                    single_packet=False,
                    queue_num=queue,
                    sbuf_tokens_per_rank=0,
                    sbuf_free_dim_per_rank=0,
                    sbuf_free_dim_pad_per_rank=0,
                    sbuf_byte_offset=0,
                ))

        def agg_pass(table, bias_row, drain, post):
            """Per tile t: psa[dst, feat] = sum_e w_e table[src_e] (+bias);
            drain(t, psa) -> ACT epilogue; post(t) deferred one tile."""
            issued = {}

            def get_call(qq, k):
                if (qq, k) not in issued:
                    c0, L = plan.call_col0[(qq, k)]
                    g = gps[qq].tile([P, cpc, d_h], BF16, tag=f"gt{qq}")
                    q0 = (qq // 2) * QRP
                    q1 = q0 + QRP
                    par = qq % 2
                    gather128(
                        g[:, :L, :],
                        table[q0:q1, par * d_h:(par + 1) * d_h],
                        idx_sb[:, c0:c0 + L * 8],
                        L, qq % nqq,
                    )
                    issued[(qq, k)] = g
                return issued[(qq, k)]

            for t in range(T):
                ops = plan.tile_ops[t]
                psa = aggp.tile([P, d_h], F32, space="PSUM", tag="agg")
                last = len(ops) - 1
                for i, (qq, j, cpcol) in enumerate(ops):
                    g = get_call(qq, j // cpc)
                    col = j % cpc
                    oh = ohp.tile([P, P], BF16, tag="oh")
                    nc.vector.tensor_scalar(
                        oh[:], iota_f[:], dl_sb[:, cpcol:cpcol + 1],
                        vl_sb[:, cpcol:cpcol + 1],
                        mybir.AluOpType.is_equal, mybir.AluOpType.mult)
                    stop = (i == last) and bias_row is None
                    nc.tensor.matmul(psa[:], lhsT=oh[:],
                                     rhs=g[:, col, :],
                                     start=(i == 0), stop=stop)
                if bias_row is not None:
                    nc.tensor.matmul(psa[:], lhsT=ones_bf[:], rhs=bias_row,
                                     start=False, stop=True)
                drain(t, psa)
                if t > 0:
                    post(t - 1)
            post(T - 1)

        # ---- pass 1: h = relu(agg + b1); store h -> t2s_own ----
        def drain1(t, psa):
            ht = hp.tile([P, d_h], BF16, tag="h")
            nc.scalar.activation(ht[:], psa[:], AF.Relu)
            hts[t] = ht

        def post1(t):
            r0 = t * P
            r1 = min(S, r0 + P)
            if r1 > r0:
                nc.sync.dma_start(t2s_own[r0:r1, :], hts[t][:r1 - r0, :])
            hts[t] = None

        hts = [None] * T
        agg_pass(t1s_full, b1r[:], drain1, post1)

        if _nocoll:
            nc.sync.dma_start(t2s_full[0:S // 2, :], t2s_own[:, :])
        else:
            nc.gpsimd.collective_compute(
                "AllGather", mybir.AluOpType.bypass, replica_groups=rg,
                ins=[t2s_own[:, :].opt()], outs=[t2s_full[:, :].opt()])

        # ---- pass 2: out = agg2 @ Wcat + bcat ----
        zts = [None] * T

        def drain2(t, psa):
            zt = zp.tile([P, d_h], F32, tag="z")
            nc.scalar.activation(zt[:], psa[:], AF.Copy)
            zts[t] = zt

        def post2(t):
            pst = mmp.tile([P, P], F32, space="PSUM", tag="mm")
            nc.tensor.transpose(pst[:d_h, :], zts[t][:], ident[:])
            ztT = zTp.tile([d_h, P], F32, tag="zT")
            nc.scalar.activation(ztT[:], pst[:d_h, :], AF.Copy)
            pso = mmp.tile([P, P], F32, space="PSUM", tag="mm")
            nc.tensor.matmul(pso[:, :d_o], lhsT=ztT[:], rhs=wc_sb[:],
                             start=True, stop=False)
            nc.tensor.matmul(pso[:, :d_o], lhsT=ones_row[:], rhs=bcr[:],
                             start=False, stop=True)
            ot = op.tile([P, d_o], F32, tag="o")
            nc.scalar.activation(ot[:], pso[:, :d_o], AF.Copy)
            nc.sync.dma_start(out_d[t * P:(t + 1) * P, :], ot[:])
            zts[t] = None

        agg_pass(t2s_full, None, drain2, post2)

    nc.compile()
    return nc


_CACHE = {}


def _get_program(n, e, d_in, d_h, d_o, n_cores, cpc, edge_key, src, dst,
                 max_slice_rows=32000):
    key = (n, e, d_in, d_h, d_o, n_cores, cpc, edge_key, max_slice_rows)
    if key not in _CACHE:
        plan = _Plan(n, n_cores, cpc, src, dst, max_slice_rows)
        nc = _build(plan, d_in, d_h, d_o)
        _CACHE[key] = (plan, nc)
    return _CACHE[key]


def kernel(x, edge_index, W1, b1, W_mu, b_mu, W_log, b_log,
           n_cores=8, cpc=16, max_slice_rows=32000, _run_kwargs=None):
    from concourse.bass_utils import run_bass_kernel_spmd

    x = np.asarray(x, np.float32)
    edge_index = np.asarray(edge_index)
    W1 = np.asarray(W1, np.float32)
    Wcat = np.concatenate([np.asarray(W_mu, np.float32),
                           np.asarray(W_log, np.float32)], axis=1)
    bcat = np.concatenate([np.asarray(b_mu, np.float32),
                           np.asarray(b_log, np.float32)])
    b1 = np.asarray(b1, np.float32)
    n, d_in = x.shape
    d_h = W1.shape[1]
    d_o = Wcat.shape[1]
    lat = np.asarray(W_mu, np.float32).shape[1]
    src = edge_index[0].astype(np.int64)
    dst = edge_index[1].astype(np.int64)

    edge_key = hash((src.tobytes(), dst.tobytes()))
    plan, nc = _get_program(n, len(src), d_in, d_h, d_o, n_cores, cpc,
                            edge_key, src, dst, max_slice_rows)

    import ml_dtypes
    bf16 = ml_dtypes.bfloat16
    in_maps = []
    for c in range(n_cores):
        idx_u, dl, vl = plan.core_arrays(c)
        xs = np.zeros((plan.SPAD, d_in), np.float32)
        xs[:plan.S] = x[c * plan.S:(c + 1) * plan.S]
        in_maps.append({
            "xT": np.ascontiguousarray(xs.T).astype(bf16),
            "w1": W1.astype(bf16), "wcat": Wcat,
            "b1r": np.ascontiguousarray(b1[None, :]).astype(bf16),
            "bcr": np.ascontiguousarray(bcat[None, :]),
            "dstloc": dl, "edgew": vl, "gidx": idx_u,
        })

    global _LAST_RESULT, _LAST_IN_MAPS
    _LAST_IN_MAPS = in_maps
    res = run_bass_kernel_spmd(nc, in_maps, core_ids=list(range(n_cores)),
                               **(_run_kwargs or {}))
    _LAST_RESULT = res
    out = np.concatenate(
        [res.results[c]["out2"][:plan.S] for c in range(n_cores)], axis=0)
    return (out[:, :lat].copy(), out[:, lat:].copy())


_LAST_RESULT = None
_LAST_IN_MAPS = None


# revision 49
# speedup vs baseline: 4.1185x; 4.1185x over previous
"""GCN VGAE encoder (3x GCNConv) on 8 Trainium2 NeuronCores — v2.

Strategy: shard nodes across 8 cores, partition edges by destination
tile, replicate weights, AllGather the projected node-feature table
between layers, gather rows per edge with dma_gather (4 SWDGE queues,
one per table quarter).

v2 math: all of A_hat's normalization is folded into per-edge weights
w_e = dinv[src]*dinv[dst] (self-loops are explicit edges with w =
dinv^2), so
    gcn1: h   = relu(sum_e w_e * (x W1)[src] + b1)        per dst
    gcn2: out = (sum_e w_e * h[src]) @ [W_mu|W_log] + b   per dst
Aggregation runs TRANSPOSED on PE: psaT[feat, dst] += matmul(
lhsT=gathered[edge, feat], rhs=onehot[edge, dst]) where onehot =
(iota == dstloc) * w is a single fused DVE op. PSUM drains go through
the (otherwise idle) Activation engine, so the DVE one-hot stream and
the PE matmul stream never block on per-tile epilogues.
"""

import os

import numpy as np

P = 128


def _ceil_div(a, b):
    return -(-a // b)


class _Plan:
    """Host-side edge partitioning shared by all cores (SPMD => one
    common chunk structure = max over cores, padded)."""

    def __init__(self, n, n_cores, cpc, src, dst, max_slice_rows=32000):
        assert n % n_cores == 0
        self.n = n
        self.n_cores = n_cores
        self.cpc = cpc                     # chunks per dma_gather call
        self.S = n // n_cores              # nodes per core
        self.T = _ceil_div(self.S, P)      # dst tiles per core
        self.SPAD = self.T * P
        # tables are packed two nodes per 256B row; edges stream in 4
        # groups (node half x parity) so each chunk reads one uniform
        # 64-col slice of its gathered pair-rows. int16 gather idx =>
        # pair rows per table slice <= 32767.
        assert n % 4 == 0 and n // 4 <= max_slice_rows
        self.NQ = 4
        self.QRP = n // 4                  # pair-rows per table slice

        # normalization weights; self-loops appended as explicit edges
        deg = np.bincount(dst, minlength=n)
        dinv = 1.0 / np.sqrt(deg + 1.0)
        loops = np.arange(n, dtype=src.dtype)
        src = np.concatenate([src, loops])
        dst = np.concatenate([dst, loops])
        w = (dinv[src] * dinv[dst]).astype(np.float32)

        core = dst // self.S
        drel = dst - core * self.S
        tt = drel // P
        loc = (drel % P).astype(np.float32)
        half = src // (n // 2)
        q = half * 2 + (src % 2)           # stream = (half, parity)
        qsrc = (src // 2 - half * self.QRP).astype(np.int16)

        T, NQ = self.T, self.NQ
        key = (core * NQ + q) * T + tt
        counts = np.bincount(key, minlength=n_cores * NQ * T).reshape(
            n_cores, NQ, T
        )
        # common run length per (quarter, tile): max over cores, runs are
        # packed back-to-back in the quarter stream (no 128-padding per
        # run; chunks may span adjacent tiles).
        self.rl = counts.max(axis=0)                         # [NQ, T]
        self.run_start = np.zeros((NQ, T), np.int64)
        self.run_start[:, 1:] = np.cumsum(self.rl, axis=1)[:, :-1]
        self.NQE = self.rl.sum(axis=1)                       # edges/quarter
        self.NQC = _ceil_div(self.NQE, P)                    # chunks/quarter
        self.NCH = int(self.NQC.sum())

        # order edges by (core, quarter, tile); rank within group
        sidx = np.lexsort((tt, q, core))
        self.sc = core[sidx]
        self.sq = q[sidx]
        self.st = tt[sidx]
        self.sqsrc = qsrc[sidx]
        self.sloc = loc[sidx]
        self.sw = w[sidx]
        gkey = (self.sc * NQ + self.sq) * T + self.st
        first = np.r_[True, gkey[1:] != gkey[:-1]]
        gstart = np.flatnonzero(first)
        glen = np.diff(np.r_[gstart, len(gkey)])
        self.rank = np.arange(len(gkey)) - np.repeat(gstart, glen)

        # chunk-part (cp) map: device consumes tiles in order; for tile t
        # and quarter q, the run covers chunks j0..j1 of quarter q's
        # stream; each (t, q, j) overlap gets its own dstloc column.
        self.tile_ops = []        # [T] -> list of (q, j, cp_col)
        ncp = 0
        for t in range(T):
            ops = []
            for qq in range(NQ):
                r0 = int(self.run_start[qq, t])
                r1 = r0 + int(self.rl[qq, t])
                if r1 == r0:
                    continue
                for j in range(r0 // P, (r1 - 1) // P + 1):
                    ops.append((qq, j, ncp))
                    ncp += 1
            self.tile_ops.append(ops)
        self.NCP = ncp
        # vectorized cp lookup: cp = cp_base[t] + ops_before[q,t] + (j - j0)
        self.cp_base = np.zeros(T, np.int64)
        run2 = 0
        self.ops_before = np.zeros((NQ, T), np.int64)
        self.j0 = self.run_start // P
        for t in range(T):
            self.cp_base[t] = run2
            acc = 0
            for qq in range(NQ):
                self.ops_before[qq, t] = acc
                if self.rl[qq, t] > 0:
                    r0 = int(self.run_start[qq, t])
                    r1 = r0 + int(self.rl[qq, t])
                    acc += (r1 - 1) // P - r0 // P + 1
            run2 += acc
        assert run2 == ncp

        # gather calls per quarter
        self.ncalls = [_ceil_div(int(c), cpc) for c in self.NQC]
        # idx tensor column offset of each (quarter, call)
        self.call_col0 = {}
        col = 0
        for qq in range(NQ):
            for k in range(self.ncalls[qq]):
                L = min(cpc, int(self.NQC[qq]) - k * cpc)
                self.call_col0[(qq, k)] = (col, L)
                col += L * 8
        self.IDXCOLS = col

    def core_arrays(self, c):
        """Per-core upload tensors: gather idx [128, IDXCOLS] i16,
        dstloc [128, NCP] f32, edge weight [128, NCP] f32."""
        NQ, cpc = self.NQ, self.cpc
        m_core = self.sc == c
        idx_out = np.zeros((P, self.IDXCOLS), np.int16)
        dl = np.full((self.NCP, P), 255.0, np.float32)
        vl = np.zeros((self.NCP, P), np.float32)
        mloc = self.sloc[m_core]
        mq = self.sq[m_core]
        mt = self.st[m_core]
        mrank = self.rank[m_core]
        msrc = self.sqsrc[m_core]
        mw = self.sw[m_core]
        # stream position of each edge within its quarter
        pos = self.run_start[mq, mt] + mrank
        cpcol = (self.cp_base[mt] + self.ops_before[mq, mt]
                 + pos // P - self.j0[mq, mt])
        dl[cpcol, pos % P] = mloc
        vl[cpcol, pos % P] = mw
        for qq in range(NQ):
            mm = mq == qq
            arr = np.zeros(int(self.NQC[qq]) * P, np.int16)
            arr[pos[mm]] = msrc[mm]
            for k in range(self.ncalls[qq]):
                c0, L = self.call_col0[(qq, k)]
                seg = arr[k * cpc * P:(k * cpc + L) * P]
                wrapped = seg.reshape(L * 8, 16).T       # [16, L*8]
                idx_out[:, c0:c0 + L * 8] = np.tile(wrapped, (8, 1))
        return idx_out, dl.T.copy(), vl.T.copy()


def _build(plan, d_in, d_h, d_o):
    """Build the SPMD Bass program (same for every core)."""
    import concourse.mybir as mybir
    import concourse.tile as tile
    from concourse import bacc
    from concourse.masks import make_identity

    F32 = mybir.dt.float32
    BF16 = mybir.dt.bfloat16
    I16 = mybir.dt.int16
    AF = mybir.ActivationFunctionType
    DPAD = 2 * d_h   # two bf16 nodes per 256B table row (gather elem size)
    n, T, NQ, SPAD, S = plan.n, plan.T, plan.NQ, plan.SPAD, plan.S
    QRP = plan.QRP
    cpc = plan.cpc
    n_cores = plan.n_cores

    nqq = int(os.environ.get("GCN_QUEUES", "4"))
    nc = bacc.Bacc("TRN2", target_bir_lowering=False,
                   debug=False, num_devices=n_cores,
                   num_swdge_queues=nqq)

    xT_d = nc.dram_tensor("xT", [d_in, SPAD], BF16, kind="ExternalInput")
    w1_d = nc.dram_tensor("w1", [d_in, d_h], BF16, kind="ExternalInput")
    wc_d = nc.dram_tensor("wcat", [d_h, d_o], F32, kind="ExternalInput")
    b1_d = nc.dram_tensor("b1r", [1, d_h], BF16, kind="ExternalInput")
    bc_d = nc.dram_tensor("bcr", [1, d_o], F32, kind="ExternalInput")
    dl_d = nc.dram_tensor("dstloc", [P, plan.NCP], F32, kind="ExternalInput")
    vl_d = nc.dram_tensor("edgew", [P, plan.NCP], F32, kind="ExternalInput")
    idx_d = nc.dram_tensor("gidx", [P, plan.IDXCOLS], I16, kind="ExternalInput")
    out_d = nc.dram_tensor("out2", [SPAD, d_o], F32, kind="ExternalOutput")

    t1s_own = nc.dram_tensor("t1s_own", [S, d_h], BF16, kind="Internal")
    t1s_full = nc.dram_tensor("t1s_full", [n // 2, DPAD], BF16,
                              kind="Internal", addr_space="Shared")
    t2s_own = nc.dram_tensor("t2s_own", [S, d_h], BF16, kind="Internal")
    t2s_full = nc.dram_tensor("t2s_full", [n // 2, DPAD], BF16,
                              kind="Internal", addr_space="Shared")
    rg = [list(range(n_cores))]
    _nocoll = bool(os.environ.get("GCN_NOCOLL"))

    from contextlib import ExitStack

    with tile.TileContext(nc, num_cores=n_cores) as tc, ExitStack() as st:
        cp = st.enter_context(tc.tile_pool(name="consts", bufs=1))
        xp = st.enter_context(tc.tile_pool(name="x", bufs=3))
        t1p = st.enter_context(tc.tile_pool(name="t1", bufs=3))
        hp = st.enter_context(tc.tile_pool(name="h", bufs=3))
        zp = st.enter_context(tc.tile_pool(name="z", bufs=3))
        zTp = st.enter_context(tc.tile_pool(name="zT", bufs=3))
        op = st.enter_context(tc.tile_pool(name="o", bufs=3))
        ohp = st.enter_context(tc.tile_pool(name="oh", bufs=8))
        gps = [st.enter_context(tc.tile_pool(name=f"g{q}", bufs=4))
               for q in range(NQ)]
        mmp = st.enter_context(tc.tile_pool(name="mm", bufs=3, space="PSUM"))
        aggp = st.enter_context(tc.tile_pool(name="agg", bufs=3,
                                             space="PSUM"))

        # ---- constants ----
        iota_i = cp.tile([P, P], mybir.dt.int32)
        nc.gpsimd.iota(iota_i[:], pattern=[[1, P]], base=0,
                       channel_multiplier=0)
        iota_f = cp.tile([P, P], BF16)
        nc.vector.tensor_copy(iota_f[:], iota_i[:])
        ident = cp.tile([P, P], F32)
        make_identity(nc, ident[:])
        ones_row = cp.tile([1, P], F32)
        nc.gpsimd.memset(ones_row[:], 1.0)
        ones_bf = cp.tile([1, P], BF16)
        nc.gpsimd.memset(ones_bf[:], 1.0)

        w1_sb = cp.tile([d_in, d_h], BF16)
        nc.sync.dma_start(w1_sb[:], w1_d[:, :])
        wc_sb = cp.tile([d_h, d_o], F32)
        nc.sync.dma_start(wc_sb[:], wc_d[:, :])
        b1r = cp.tile([1, d_h], BF16)
        nc.sync.dma_start(b1r[:], b1_d[:, :])
        bcr = cp.tile([1, d_o], F32)
        nc.sync.dma_start(bcr[:], bc_d[:, :])

        dl_sb = cp.tile([P, plan.NCP], F32)
        nc.sync.dma_start(dl_sb[:], dl_d[:, :])
        vl_sb = cp.tile([P, plan.NCP], F32)
        nc.sync.dma_start(vl_sb[:], vl_d[:, :])
        idx_sb = cp.tile([P, plan.IDXCOLS], I16)
        nc.sync.dma_start(idx_sb[:], idx_d[:, :])

        # ---- layer-1 projection: t1s_own = x @ W1 (raw, no scaling) ----
        for t in range(T):
            xt = xp.tile([d_in, P], BF16, tag="x")
            nc.sync.dma_start(xt[:], xT_d[:, t * P:(t + 1) * P])
            psm = mmp.tile([P, P], F32, space="PSUM", tag="mm")
            nc.tensor.matmul(psm[:, :d_h], lhsT=xt[:], rhs=w1_sb[:],
                             start=True, stop=True)
            t1t = t1p.tile([P, d_h], BF16, tag="t1")
            nc.scalar.activation(t1t[:], psm[:, :d_h], AF.Copy)
            r0 = t * P
            r1 = min(S, r0 + P)
            if r1 > r0:
                nc.sync.dma_start(t1s_own[r0:r1, :], t1t[:r1 - r0, :])

        if _nocoll:
            nc.sync.dma_start(t1s_full[0:S // 2, :], t1s_own[:, :])
        else:
            nc.gpsimd.collective_compute(
                "AllGather", mybir.AluOpType.bypass, replica_groups=rg,
                ins=[t1s_own[:, :].opt()], outs=[t1s_full[:, :].opt()])

        def agg_pass(table, bias_row, drain, post):
            """Per tile t: psa[dst, feat] = sum_e w_e table[src_e] (+bias);
            drain(t, psa) -> ACT epilogue; post(t) deferred one tile."""
            issued = {}

            def get_call(qq, k):
                if (qq, k) not in issued:
                    c0, L = plan.call_col0[(qq, k)]
                    g = gps[qq].tile([P, cpc, DPAD], BF16, tag=f"gt{qq}")
                    q0 = (qq // 2) * QRP
                    q1 = q0 + QRP
                    nc.gpsimd.dma_gather(
                        out_ap=g[:, :L, :],
                        in_ap=table[q0:q1, :],
                        idxs_ap=idx_sb[:, c0:c0 + L * 8],
                        num_idxs=L * P,
                        num_idxs_reg=L * P,
                        elem_size=DPAD,
                        single_packet=bool(os.environ.get("GCN_SP")),
                        queue_num=qq % nqq,
                    )
                    issued[(qq, k)] = g
                return issued[(qq, k)]

            for t in range(T):
                ops = plan.tile_ops[t]
                psa = aggp.tile([P, d_h], F32, space="PSUM", tag="agg")
                last = len(ops) - 1
                for i, (qq, j, cpcol) in enumerate(ops):
                    g = get_call(qq, j // cpc)
                    col = j % cpc
                    oh = ohp.tile([P, P], BF16, tag="oh")
                    nc.vector.tensor_scalar(
                        oh[:], iota_f[:], dl_sb[:, cpcol:cpcol + 1],
                        vl_sb[:, cpcol:cpcol + 1],
                        mybir.AluOpType.is_equal, mybir.AluOpType.mult)
                    stop = (i == last) and bias_row is None
                    par = qq % 2
                    nc.tensor.matmul(psa[:], lhsT=oh[:],
                                     rhs=g[:, col, par * d_h:
                                           (par + 1) * d_h],
                                     start=(i == 0), stop=stop)
                if bias_row is not None:
                    nc.tensor.matmul(psa[:], lhsT=ones_bf[:], rhs=bias_row,
                                     start=False, stop=True)
                drain(t, psa)
                if t > 0:
                    post(t - 1)
            post(T - 1)

        # ---- pass 1: h = relu(agg + b1); store h -> t2s_own ----
        def drain1(t, psa):
            ht = hp.tile([P, d_h], BF16, tag="h")
            nc.scalar.activation(ht[:], psa[:], AF.Relu)
            hts[t] = ht

        def post1(t):
            r0 = t * P
            r1 = min(S, r0 + P)
            if r1 > r0:
                nc.sync.dma_start(t2s_own[r0:r1, :], hts[t][:r1 - r0, :])
            hts[t] = None

        hts = [None] * T
        agg_pass(t1s_full, b1r[:], drain1, post1)

        if _nocoll:
            nc.sync.dma_start(t2s_full[0:S // 2, :], t2s_own[:, :])
        else:
            nc.gpsimd.collective_compute(
                "AllGather", mybir.AluOpType.bypass, replica_groups=rg,
                ins=[t2s_own[:, :].opt()], outs=[t2s_full[:, :].opt()])

        # ---- pass 2: out = agg2 @ Wcat + bcat ----
        zts = [None] * T

        def drain2(t, psa):
            zt = zp.tile([P, d_h], F32, tag="z")
            nc.scalar.activation(zt[:], psa[:], AF.Copy)
            zts[t] = zt

        def post2(t):
            pst = mmp.tile([P, P], F32, space="PSUM", tag="mm")
            nc.tensor.transpose(pst[:d_h, :], zts[t][:], ident[:])
            ztT = zTp.tile([d_h, P], F32, tag="zT")
            nc.scalar.activation(ztT[:], pst[:d_h, :], AF.Copy)
            pso = mmp.tile([P, P], F32, space="PSUM", tag="mm")
            nc.tensor.matmul(pso[:, :d_o], lhsT=ztT[:], rhs=wc_sb[:],
                             start=True, stop=False)
            nc.tensor.matmul(pso[:, :d_o], lhsT=ones_row[:], rhs=bcr[:],
                             start=False, stop=True)
            ot = op.tile([P, d_o], F32, tag="o")
            nc.scalar.activation(ot[:], pso[:, :d_o], AF.Copy)
            nc.sync.dma_start(out_d[t * P:(t + 1) * P, :], ot[:])
            zts[t] = None

        agg_pass(t2s_full, None, drain2, post2)

    nc.compile()
    return nc


_CACHE = {}


def _get_program(n, e, d_in, d_h, d_o, n_cores, cpc, edge_key, src, dst,
                 max_slice_rows=32000):
    key = (n, e, d_in, d_h, d_o, n_cores, cpc, edge_key, max_slice_rows)
    if key not in _CACHE:
        plan = _Plan(n, n_cores, cpc, src, dst, max_slice_rows)
        nc = _build(plan, d_in, d_h, d_o)
        _CACHE[key] = (plan, nc)
    return _CACHE[key]


def kernel(x, edge_index, W1, b1, W_mu, b_mu, W_log, b_log,
           n_cores=8, cpc=16, max_slice_rows=32000, _run_kwargs=None):
    from concourse.bass_utils import run_bass_kernel_spmd

    x = np.asarray(x, np.float32)
    edge_index = np.asarray(edge_index)
    W1 = np.asarray(W1, np.float32)
    Wcat = np.concatenate([np.asarray(W_mu, np.float32),
                           np.asarray(W_log, np.float32)], axis=1)
    bcat = np.concatenate([np.asarray(b_mu, np.float32),
                           np.asarray(b_log, np.float32)])
    b1 = np.asarray(b1, np.float32)
    n, d_in = x.shape
    d_h = W1.shape[1]
    d_o = Wcat.shape[1]
    lat = np.asarray(W_mu, np.float32).shape[1]
    src = edge_index[0].astype(np.int64)
    dst = edge_index[1].astype(np.int64)

    edge_key = hash((src.tobytes(), dst.tobytes()))
    plan, nc = _get_program(n, len(src), d_in, d_h, d_o, n_cores, cpc,
                            edge_key, src, dst, max_slice_rows)

    import ml_dtypes
    bf16 = ml_dtypes.bfloat16
    in_maps = []
    for c in range(n_cores):
        idx_u, dl, vl = plan.core_arrays(c)
        xs = np.zeros((plan.SPAD, d_in), np.float32)
        xs[:plan.S] = x[c * plan.S:(c + 1) * plan.S]
        in_maps.append({
            "xT": np.ascontiguousarray(xs.T).astype(bf16),
            "w1": W1.astype(bf16), "wcat": Wcat,
            "b1r": np.ascontiguousarray(b1[None, :]).astype(bf16),
            "bcr": np.ascontiguousarray(bcat[None, :]),
            "dstloc": dl, "edgew": vl, "gidx": idx_u,
        })

    global _LAST_RESULT, _LAST_IN_MAPS
    _LAST_IN_MAPS = in_maps
    res = run_bass_kernel_spmd(nc, in_maps, core_ids=list(range(n_cores)),
                               **(_run_kwargs or {}))
    _LAST_RESULT = res
    out = np.concatenate(
        [res.results[c]["out2"][:plan.S] for c in range(n_cores)], axis=0)
    return (out[:, :lat].copy(), out[:, lat:].copy())


_LAST_RESULT = None
_LAST_IN_MAPS = None


# revision 58
# speedup vs baseline: 4.2877x; 1.0411x over previous
"""GCN VGAE encoder (3x GCNConv) on 8 Trainium2 NeuronCores — v2.

Strategy: shard nodes across 8 cores, partition edges by destination
tile, replicate weights, AllGather the projected node-feature table
between layers, gather rows per edge with dma_gather (4 SWDGE queues,
one per table quarter).

v2 math: all of A_hat's normalization is folded into per-edge weights
w_e = dinv[src]*dinv[dst] (self-loops are explicit edges with w =
dinv^2), so
    gcn1: h   = relu(sum_e w_e * (x W1)[src] + b1)        per dst
    gcn2: out = (sum_e w_e * h[src]) @ [W_mu|W_log] + b   per dst
Aggregation runs TRANSPOSED on PE: psaT[feat, dst] += matmul(
lhsT=gathered[edge, feat], rhs=onehot[edge, dst]) where onehot =
(iota == dstloc) * w is a single fused DVE op. PSUM drains go through
the (otherwise idle) Activation engine, so the DVE one-hot stream and
the PE matmul stream never block on per-tile epilogues.
"""

import os

import numpy as np

P = 128


def _ceil_div(a, b):
    return -(-a // b)


class _Plan:
    """Host-side edge partitioning shared by all cores (SPMD => one
    common chunk structure = max over cores, padded)."""

    def __init__(self, n, n_cores, cpc, src, dst, max_slice_rows=32000):
        assert n % n_cores == 0
        self.n = n
        self.n_cores = n_cores
        self.cpc = cpc                     # chunks per dma_gather call
        self.S = n // n_cores              # nodes per core
        self.T = _ceil_div(self.S, P)      # dst tiles per core
        self.SPAD = self.T * P
        # tables are packed two nodes per 256B row; edges stream in 4
        # groups (node half x parity) so each chunk reads one uniform
        # 64-col slice of its gathered pair-rows. int16 gather idx =>
        # pair rows per table slice <= 32767.
        assert n % 4 == 0 and n // 4 <= max_slice_rows
        self.NQ = 4
        self.QRP = n // 4                  # pair-rows per table slice

        # normalization weights; self-loops appended as explicit edges
        deg = np.bincount(dst, minlength=n)
        dinv = 1.0 / np.sqrt(deg + 1.0)
        loops = np.arange(n, dtype=src.dtype)
        src = np.concatenate([src, loops])
        dst = np.concatenate([dst, loops])
        w = (dinv[src] * dinv[dst]).astype(np.float32)

        core = dst // self.S
        drel = dst - core * self.S
        tt = drel // P
        loc = (drel % P).astype(np.float32)
        # table position: half h = which half of the owner's rows, so the
        # AllGather can ship in two halves (h=0 usable before h=1 lands).
        S2 = self.S // 2
        assert S2 % 2 == 0
        c_s = src // self.S
        p_s = src % self.S
        h = p_s // S2
        p = p_s % S2
        tnode = h * (n // 2) + c_s * S2 + p
        q = h * 2 + (p % 2)                # stream = (half, parity)
        qsrc = (tnode // 2 - h * self.QRP).astype(np.int16)

        T, NQ = self.T, self.NQ
        key = (core * NQ + q) * T + tt
        counts = np.bincount(key, minlength=n_cores * NQ * T).reshape(
            n_cores, NQ, T
        )
        # common run length per (quarter, tile): max over cores, runs are
        # packed back-to-back in the quarter stream (no 128-padding per
        # run; chunks may span adjacent tiles).
        self.rl = counts.max(axis=0)                         # [NQ, T]
        self.run_start = np.zeros((NQ, T), np.int64)
        self.run_start[:, 1:] = np.cumsum(self.rl, axis=1)[:, :-1]
        self.NQE = self.rl.sum(axis=1)                       # edges/quarter
        self.NQC = _ceil_div(self.NQE, P)                    # chunks/quarter
        self.NCH = int(self.NQC.sum())

        # order edges by (core, quarter, tile); rank within group
        sidx = np.lexsort((tt, q, core))
        self.sc = core[sidx]
        self.sq = q[sidx]
        self.st = tt[sidx]
        self.sqsrc = qsrc[sidx]
        self.sloc = loc[sidx]
        self.sw = w[sidx]
        gkey = (self.sc * NQ + self.sq) * T + self.st
        first = np.r_[True, gkey[1:] != gkey[:-1]]
        gstart = np.flatnonzero(first)
        glen = np.diff(np.r_[gstart, len(gkey)])
        self.rank = np.arange(len(gkey)) - np.repeat(gstart, glen)

        # chunk-part (cp) map: device consumes tiles in order; for tile t
        # and quarter q, the run covers chunks j0..j1 of quarter q's
        # stream; each (t, q, j) overlap gets its own dstloc column.
        self.tile_ops = []        # [T] -> list of (q, j, cp_col)
        ncp = 0
        for t in range(T):
            ops = []
            for qq in range(NQ):
                r0 = int(self.run_start[qq, t])
                r1 = r0 + int(self.rl[qq, t])
                if r1 == r0:
                    continue
                for j in range(r0 // P, (r1 - 1) // P + 1):
                    ops.append((qq, j, ncp))
                    ncp += 1
            self.tile_ops.append(ops)
        self.NCP = ncp
        # vectorized cp lookup: cp = cp_base[t] + ops_before[q,t] + (j - j0)
        self.cp_base = np.zeros(T, np.int64)
        run2 = 0
        self.ops_before = np.zeros((NQ, T), np.int64)
        self.j0 = self.run_start // P
        for t in range(T):
            self.cp_base[t] = run2
            acc = 0
            for qq in range(NQ):
                self.ops_before[qq, t] = acc
                if self.rl[qq, t] > 0:
                    r0 = int(self.run_start[qq, t])
                    r1 = r0 + int(self.rl[qq, t])
                    acc += (r1 - 1) // P - r0 // P + 1
            run2 += acc
        assert run2 == ncp

        # gather calls per quarter
        self.ncalls = [_ceil_div(int(c), cpc) for c in self.NQC]
        # idx tensor column offset of each (quarter, call)
        self.call_col0 = {}
        col = 0
        for qq in range(NQ):
            for k in range(self.ncalls[qq]):
                L = min(cpc, int(self.NQC[qq]) - k * cpc)
                self.call_col0[(qq, k)] = (col, L)
                col += L * 8
        self.IDXCOLS = col

    def core_arrays(self, c):
        """Per-core upload tensors: gather idx [128, IDXCOLS] i16,
        dstloc [128, NCP] f32, edge weight [128, NCP] f32."""
        NQ, cpc = self.NQ, self.cpc
        m_core = self.sc == c
        idx_out = np.zeros((P, self.IDXCOLS), np.int16)
        dl = np.full((self.NCP, P), 255.0, np.float32)
        vl = np.zeros((self.NCP, P), np.float32)
        mloc = self.sloc[m_core]
        mq = self.sq[m_core]
        mt = self.st[m_core]
        mrank = self.rank[m_core]
        msrc = self.sqsrc[m_core]
        mw = self.sw[m_core]
        # stream position of each edge within its quarter
        pos = self.run_start[mq, mt] + mrank
        cpcol = (self.cp_base[mt] + self.ops_before[mq, mt]
                 + pos // P - self.j0[mq, mt])
        dl[cpcol, pos % P] = mloc
        vl[cpcol, pos % P] = mw
        for qq in range(NQ):
            mm = mq == qq
            arr = np.zeros(int(self.NQC[qq]) * P, np.int16)
            arr[pos[mm]] = msrc[mm]
            for k in range(self.ncalls[qq]):
                c0, L = self.call_col0[(qq, k)]
                seg = arr[k * cpc * P:(k * cpc + L) * P]
                wrapped = seg.reshape(L * 8, 16).T       # [16, L*8]
                idx_out[:, c0:c0 + L * 8] = np.tile(wrapped, (8, 1))
        return idx_out, dl.T.copy(), vl.T.copy()


def _build(plan, d_in, d_h, d_o):
    """Build the SPMD Bass program (same for every core)."""
    import concourse.mybir as mybir
    import concourse.tile as tile
    from concourse import bacc
    from concourse.masks import make_identity

    F32 = mybir.dt.float32
    BF16 = mybir.dt.bfloat16
    I16 = mybir.dt.int16
    AF = mybir.ActivationFunctionType
    DPAD = 2 * d_h   # two bf16 nodes per 256B table row (gather elem size)
    n, T, NQ, SPAD, S = plan.n, plan.T, plan.NQ, plan.SPAD, plan.S
    QRP = plan.QRP
    cpc = plan.cpc
    n_cores = plan.n_cores

    nqq = int(os.environ.get("GCN_QUEUES", "4"))
    nc = bacc.Bacc("TRN2", target_bir_lowering=False,
                   debug=False, num_devices=n_cores,
                   num_swdge_queues=nqq)

    xT_d = nc.dram_tensor("xT", [d_in, SPAD], BF16, kind="ExternalInput")
    w1_d = nc.dram_tensor("w1", [d_in, d_h], BF16, kind="ExternalInput")
    wc_d = nc.dram_tensor("wcat", [d_h, d_o], F32, kind="ExternalInput")
    b1_d = nc.dram_tensor("b1r", [1, d_h], BF16, kind="ExternalInput")
    bc_d = nc.dram_tensor("bcr", [1, d_o], F32, kind="ExternalInput")
    dl_d = nc.dram_tensor("dstloc", [P, plan.NCP], F32, kind="ExternalInput")
    vl_d = nc.dram_tensor("edgew", [P, plan.NCP], F32, kind="ExternalInput")
    idx_d = nc.dram_tensor("gidx", [P, plan.IDXCOLS], I16, kind="ExternalInput")
    out_d = nc.dram_tensor("out2", [SPAD, d_o], F32, kind="ExternalOutput")

    t1s_own = nc.dram_tensor("t1s_own", [S, d_h], BF16, kind="Internal")
    t1s_full = nc.dram_tensor("t1s_full", [n // 2, DPAD], BF16,
                              kind="Internal", addr_space="Shared")
    t2s_own = nc.dram_tensor("t2s_own", [S, d_h], BF16, kind="Internal")
    t2s_full = nc.dram_tensor("t2s_full", [n // 2, DPAD], BF16,
                              kind="Internal", addr_space="Shared")
    rg = [list(range(n_cores))]
    _nocoll = bool(os.environ.get("GCN_NOCOLL"))
    S2 = S // 2

    from contextlib import ExitStack

    with tile.TileContext(nc, num_cores=n_cores) as tc, ExitStack() as st:
        cp = st.enter_context(tc.tile_pool(name="consts", bufs=1))
        bigp = st.enter_context(tc.tile_pool(name="big", bufs=1))
        xp = st.enter_context(tc.tile_pool(name="x", bufs=3))
        t1p = st.enter_context(tc.tile_pool(name="t1", bufs=3))
        hp = st.enter_context(tc.tile_pool(name="h", bufs=3))
        zp = st.enter_context(tc.tile_pool(name="z", bufs=3))
        zTp = st.enter_context(tc.tile_pool(name="zT", bufs=3))
        op = st.enter_context(tc.tile_pool(name="o", bufs=3))
        ohp = st.enter_context(tc.tile_pool(name="oh", bufs=8))
        gps = [st.enter_context(tc.tile_pool(name=f"g{q}", bufs=4))
               for q in range(NQ)]
        mmp = st.enter_context(tc.tile_pool(name="mm", bufs=3, space="PSUM"))
        aggp = st.enter_context(tc.tile_pool(name="agg", bufs=3,
                                             space="PSUM"))

        # ---- constants ----
        iota_i = cp.tile([P, P], mybir.dt.int32)
        nc.gpsimd.iota(iota_i[:], pattern=[[1, P]], base=0,
                       channel_multiplier=0)
        iota_f = cp.tile([P, P], BF16)
        nc.vector.tensor_copy(iota_f[:], iota_i[:])
        ident = cp.tile([P, P], F32)
        make_identity(nc, ident[:])
        ident_bf = cp.tile([P, P], BF16)
        nc.vector.tensor_copy(ident_bf[:], ident[:])
        ones_row = cp.tile([1, P], F32)
        nc.gpsimd.memset(ones_row[:], 1.0)
        ones_bf = cp.tile([1, P], BF16)
        nc.gpsimd.memset(ones_bf[:], 1.0)

        w1_sb = cp.tile([d_in, d_h], BF16)
        nc.sync.dma_start(w1_sb[:], w1_d[:, :])
        wc_sb = cp.tile([d_h, d_o], F32)
        nc.sync.dma_start(wc_sb[:], wc_d[:, :])
        b1r = cp.tile([1, d_h], BF16)
        nc.sync.dma_start(b1r[:], b1_d[:, :])
        bcr = cp.tile([1, d_o], F32)
        nc.sync.dma_start(bcr[:], bc_d[:, :])

        dl_sb = cp.tile([P, plan.NCP], F32)
        nc.sync.dma_start(dl_sb[:], dl_d[:, :])
        vl_sb = cp.tile([P, plan.NCP], F32)
        nc.sync.dma_start(vl_sb[:], vl_d[:, :])
        idx_sb = cp.tile([P, plan.IDXCOLS], I16)
        nc.sync.dma_start(idx_sb[:], idx_d[:, :])

        def emit_coll(own, full, h):
            """AllGather half h: own rows [h*S2,(h+1)*S2) -> full pair
            rows [h*QRP,(h+1)*QRP)."""
            if _nocoll:
                nc.sync.dma_start(full[h * QRP:h * QRP + S2 // 2, :],
                                  own[h * S2:(h + 1) * S2, :])
            else:
                nc.gpsimd.collective_compute(
                    "AllGather", mybir.AluOpType.bypass, replica_groups=rg,
                    ins=[own[h * S2:(h + 1) * S2, :].opt()],
                    outs=[full[h * QRP:(h + 1) * QRP, :].opt()])

        # ---- layer-1 projection: t1s_own = x @ W1 (raw, no scaling) ----
        for t in range(T):
            xt = xp.tile([d_in, P], BF16, tag="x")
            nc.sync.dma_start(xt[:], xT_d[:, t * P:(t + 1) * P])
            psm = mmp.tile([P, P], F32, space="PSUM", tag="mm")
            nc.tensor.matmul(psm[:, :d_h], lhsT=xt[:], rhs=w1_sb[:],
                             start=True, stop=True)
            t1t = t1p.tile([P, d_h], BF16, tag="t1")
            nc.scalar.activation(t1t[:], psm[:, :d_h], AF.Copy)
            r0 = t * P
            r1 = min(S, r0 + P)
            if r1 > r0:
                nc.sync.dma_start(t1s_own[r0:r1, :], t1t[:r1 - r0, :])
            if r1 >= S2 > r0:
                emit_coll(t1s_own, t1s_full, 0)
        emit_coll(t1s_own, t1s_full, 1)

        def agg_pass(table, bias_row, drain, post, mid=None):
            """Two sweeps per tile t: sweep A accumulates streams of table
            half 0 into an SBUF acc tile; sweep B re-adds it into PSUM via
            an identity matmul, adds half-1 streams (+bias), then drain(t)
            on ACT and post(t) deferred one tile. mid=(t, fn) emits fn (a
            collective) during sweep B."""
            issued = {}

            def get_call(qq, k):
                if (qq, k) not in issued:
                    c0, L = plan.call_col0[(qq, k)]
                    g = gps[qq].tile([P, cpc, DPAD], BF16, tag=f"gt{qq}")
                    q0 = (qq // 2) * QRP
                    q1 = q0 + QRP
                    nc.gpsimd.dma_gather(
                        out_ap=g[:, :L, :],
                        in_ap=table[q0:q1, :],
                        idxs_ap=idx_sb[:, c0:c0 + L * 8],
                        num_idxs=L * P,
                        num_idxs_reg=L * P,
                        elem_size=DPAD,
                        single_packet=False,
                        queue_num=(qq + 2 * k) % nqq,
                    )
                    issued[(qq, k)] = g
                return issued[(qq, k)]

            def chunk_mm(psa, qq, j, cpcol, start, stop):
                g = get_call(qq, j // cpc)
                col = j % cpc
                oh = ohp.tile([P, P], BF16, tag="oh")
                nc.vector.tensor_scalar(
                    oh[:], iota_f[:], dl_sb[:, cpcol:cpcol + 1],
                    vl_sb[:, cpcol:cpcol + 1],
                    mybir.AluOpType.is_equal, mybir.AluOpType.mult)
                par = qq % 2
                nc.tensor.matmul(psa[:], lhsT=oh[:],
                                 rhs=g[:, col, par * d_h:(par + 1) * d_h],
                                 start=start, stop=stop)

            acc = bigp.tile([P, T, d_h], BF16, tag="acc")
            for t in range(T):
                ops = [o for o in plan.tile_ops[t] if o[0] < 2]
                if not ops:
                    nc.gpsimd.memset(acc[:, t, :], 0.0)
                    continue
                psa = aggp.tile([P, d_h], F32, space="PSUM", tag="agg")
                for i, (qq, j, cpcol) in enumerate(ops):
                    chunk_mm(psa, qq, j, cpcol, i == 0, i == len(ops) - 1)
                nc.scalar.activation(acc[:, t, :], psa[:], AF.Copy)

            for t in range(T):
                ops = [o for o in plan.tile_ops[t] if o[0] >= 2]
                psa = aggp.tile([P, d_h], F32, space="PSUM", tag="agg")
                nc.tensor.matmul(psa[:], lhsT=ident_bf[:],
                                 rhs=acc[:, t, :], start=True,
                                 stop=(not ops) and bias_row is None)
                for i, (qq, j, cpcol) in enumerate(ops):
                    chunk_mm(psa, qq, j, cpcol, False,
                             (i == len(ops) - 1) and bias_row is None)
                if bias_row is not None:
                    nc.tensor.matmul(psa[:], lhsT=ones_bf[:], rhs=bias_row,
                                     start=False, stop=True)
                drain(t, psa)
                if t > 0:
                    post(t - 1)
                if mid is not None and t == mid[0]:
                    mid[1]()
            post(T - 1)

        # ---- pass 1: h = relu(agg + b1); store h -> t2s_own ----
        def drain1(t, psa):
            ht = hp.tile([P, d_h], BF16, tag="h")
            nc.scalar.activation(ht[:], psa[:], AF.Relu)
            hts[t] = ht

        def post1(t):
            r0 = t * P
            r1 = min(S, r0 + P)
            if r1 > r0:
                nc.sync.dma_start(t2s_own[r0:r1, :], hts[t][:r1 - r0, :])
            hts[t] = None

        hts = [None] * T
        mid_t = min((S2 - 1) // P + 1, T - 1)
        agg_pass(t1s_full, b1r[:], drain1, post1,
                 mid=(mid_t, lambda: emit_coll(t2s_own, t2s_full, 0)))
        emit_coll(t2s_own, t2s_full, 1)

        # ---- pass 2: out = agg2 @ Wcat + bcat ----
        zts = [None] * T

        def drain2(t, psa):
            zt = zp.tile([P, d_h], F32, tag="z")
            nc.scalar.activation(zt[:], psa[:], AF.Copy)
            zts[t] = zt

        def post2(t):
            pst = mmp.tile([P, P], F32, space="PSUM", tag="mm")
            nc.tensor.transpose(pst[:d_h, :], zts[t][:], ident[:])
            ztT = zTp.tile([d_h, P], F32, tag="zT")
            nc.scalar.activation(ztT[:], pst[:d_h, :], AF.Copy)
            pso = mmp.tile([P, P], F32, space="PSUM", tag="mm")
            nc.tensor.matmul(pso[:, :d_o], lhsT=ztT[:], rhs=wc_sb[:],
                             start=True, stop=False)
            nc.tensor.matmul(pso[:, :d_o], lhsT=ones_row[:], rhs=bcr[:],
                             start=False, stop=True)
            ot = op.tile([P, d_o], F32, tag="o")
            nc.scalar.activation(ot[:], pso[:, :d_o], AF.Copy)
            nc.sync.dma_start(out_d[t * P:(t + 1) * P, :], ot[:])
            zts[t] = None

        agg_pass(t2s_full, None, drain2, post2)

    nc.compile()
    return nc


_CACHE = {}


def _get_program(n, e, d_in, d_h, d_o, n_cores, cpc, edge_key, src, dst,
                 max_slice_rows=32000):
    key = (n, e, d_in, d_h, d_o, n_cores, cpc, edge_key, max_slice_rows)
    if key not in _CACHE:
        plan = _Plan(n, n_cores, cpc, src, dst, max_slice_rows)
        nc = _build(plan, d_in, d_h, d_o)
        _CACHE[key] = (plan, nc)
    return _CACHE[key]


def kernel(x, edge_index, W1, b1, W_mu, b_mu, W_log, b_log,
           n_cores=8, cpc=16, max_slice_rows=32000, _run_kwargs=None):
    from concourse.bass_utils import run_bass_kernel_spmd

    x = np.asarray(x, np.float32)
    edge_index = np.asarray(edge_index)
    W1 = np.asarray(W1, np.float32)
    Wcat = np.concatenate([np.asarray(W_mu, np.float32),
                           np.asarray(W_log, np.float32)], axis=1)
    bcat = np.concatenate([np.asarray(b_mu, np.float32),
                           np.asarray(b_log, np.float32)])
    b1 = np.asarray(b1, np.float32)
    n, d_in = x.shape
    d_h = W1.shape[1]
    d_o = Wcat.shape[1]
    lat = np.asarray(W_mu, np.float32).shape[1]
    src = edge_index[0].astype(np.int64)
    dst = edge_index[1].astype(np.int64)

    edge_key = hash((src.tobytes(), dst.tobytes()))
    plan, nc = _get_program(n, len(src), d_in, d_h, d_o, n_cores, cpc,
                            edge_key, src, dst, max_slice_rows)

    import ml_dtypes
    bf16 = ml_dtypes.bfloat16
    in_maps = []
    for c in range(n_cores):
        idx_u, dl, vl = plan.core_arrays(c)
        xs = np.zeros((plan.SPAD, d_in), np.float32)
        xs[:plan.S] = x[c * plan.S:(c + 1) * plan.S]
        in_maps.append({
            "xT": np.ascontiguousarray(xs.T).astype(bf16),
            "w1": W1.astype(bf16), "wcat": Wcat,
            "b1r": np.ascontiguousarray(b1[None, :]).astype(bf16),
            "bcr": np.ascontiguousarray(bcat[None, :]),
            "dstloc": dl, "edgew": vl, "gidx": idx_u,
        })

    global _LAST_RESULT, _LAST_IN_MAPS
    _LAST_IN_MAPS = in_maps
    res = run_bass_kernel_spmd(nc, in_maps, core_ids=list(range(n_cores)),
                               **(_run_kwargs or {}))
    _LAST_RESULT = res
    out = np.concatenate(
        [res.results[c]["out2"][:plan.S] for c in range(n_cores)], axis=0)
    return (out[:, :lat].copy(), out[:, lat:].copy())


_LAST_RESULT = None
_LAST_IN_MAPS = None
